# revision 8
# baseline (speedup 1.0000x reference)
"""FP8DynamicLinear Trainium2 kernel (8-core SPMD).

Reference semantics (per nn_FP8DynamicLinear):
    xb      = bf16(x)
    amax    = max(|min(xb)|, |max(xb)|)            # per-tensor, fp32
    scale   = 448 / amax
    qx      = e4m3fn(clip(xb * scale, +-448))
    a       = bf16(qx) * bf16(1/scale)             # bf16
    b       = bf16(weight) * bf16(weight_scale)    # bf16, weight already on fp8 grid
    out     = a @ b.T + bias                       # bf16 [B, S, N]

Kernel strategy (M-sharded, column of tokens per core):
  * Host: transpose x -> xT [K, M] fp32 and slice M across 8 cores;
    transpose weight and pre-cast (w.T * 0.5) to TRN fp8e4 (max 240 ==
    e4m3fn grid / 2, exact for all normals); the 2x is folded back into
    the output scale.  TRN fp8e4 differs from OCP e4m3fn only in range
    (240 vs 448), so quantizing x at scale/2 and dequantizing by 2/scale
    reproduces e4m3fn rounding bit-exactly for all |v| >= 2^-6.
  * Device, per core: stream own xT slice, cast bf16, absmax-reduce;
    AllReduce(max) across the 8 cores; quantize x*(scale/2) -> fp8 via
    ACT; fp8 DoubleRow matmul (qx.T @ wq) accumulating fp32 in PSUM;
    psum * (4 * bf16(1/scale) * bf16(w_scale)) + bias -> bf16 out.
"""

import math
from contextlib import ExitStack

import numpy as np
import ml_dtypes

import concourse.bass as bass
import concourse.mybir as mybir
import concourse.tile as tile
from concourse.bass import ts, ds
from concourse.bass_utils import run_bass_kernel_spmd
from concourse.kernels.tile_matmul import (
    ShapeInfo,
    composable_matmul_tile_kernel,
    dma_from_dram_kxn,
    dma_to_dram_mxn,
)

P = 128
F8_MAX = 448.0
N_CORES = 8

# Problem shapes (hardcoded per spec)
B, S, K, N = 2, 4096, 4096, 16384
M = B * S              # 8192 tokens
M_C = M // N_CORES     # 1024 tokens per core
K_TILE = 512
K_SUB = K_TILE // P    # 4
K_TILES = K // K_TILE  # 8
KS_TOT = K // P        # 32

_F32 = mybir.dt.float32
_BF16 = mybir.dt.bfloat16
_F8 = mybir.dt.float8e4

# ----------------------------------------------------------------------------
# Wait legalizer: this container's walrus rejects engine instructions with
# more than 1 inline sync-wait (and EventSemaphore with more than 2), but
# bass_rust nop-fusion fuses 2 waits + 1 update into one instruction.  Split
# the excess onto preceding InstEventSemaphore carriers on the same engine.
_EXEMPT = (
    "InstEventSemaphore",
    "InstUnconditionalBranch",
    "InstConditionalBranch",
    "InstCall",
    "InstRegisterMove",
)
_ES_CAP = 2


def _legalize_waits(nc, max_inline=1):
    n_split = 0
    for fn in nc.m.functions:
        for blk in fn.blocks:
            insts = blk.instructions
            i = 0
            while i < len(insts):
                inst = insts[i]
                si = inst.sync_info
                if (
                    si is not None
                    and len(si.on_wait) > max_inline
                    and inst.__class__.__name__ not in _EXEMPT
                ):
                    extra = list(si.on_wait[:-max_inline])
                    keep = list(si.on_wait[-max_inline:])
                    inst.sync_info = mybir.SyncInfo(
                        on_wait=keep, on_update=list(si.on_update)
                    )
                    carriers = [
                        mybir.InstEventSemaphore(
                            name=f"{inst.name}-wsplit{j}",
                            engine=inst.engine,
                            bass_nofuse=True,
                            sync_info=mybir.SyncInfo(
                                on_wait=extra[j : j + _ES_CAP], on_update=[]
                            ),
                        )
                        for j in range(0, len(extra), _ES_CAP)
                    ]
                    for kk, es in enumerate(carriers):
                        insts.insert(i + kk, es)
                    i += len(carriers)
                    n_split += 1
                i += 1
    return n_split


# ----------------------------------------------------------------------------
def build_nc(m_c=M_C, k=K, n=N):
    ks_tot = k // P
    k_tiles = k // K_TILE

    nc = bass.Bass()
    xT = nc.dram_tensor("xT", [k, m_c], _F32, kind="ExternalInput")
    wq = nc.dram_tensor("wq", [k, n], _F8, kind="ExternalInput")
    bias_d = nc.dram_tensor("bias", [n], _BF16, kind="ExternalInput")
    wscale_d = nc.dram_tensor("wscale", [1], _F32, kind="ExternalInput")
    out_d = nc.dram_tensor("out", [m_c, n], _BF16, kind="ExternalOutput")

    xT3 = xT.rearrange("(o p) m -> p o m", p=P)  # k = o*128 + p

    with tile.TileContext(nc) as tc:
        with ExitStack() as ctx:
            const = ctx.enter_context(tc.tile_pool(name="const", bufs=1))
            dram = ctx.enter_context(tc.tile_pool(name="dram", bufs=1, space="DRAM"))

            qx_blocks = [
                const.tile([P, K_SUB, m_c], _F8, name=f"qx{kt}", tag=f"qx{kt}")
                for kt in range(k_tiles)
            ]
            bias_bc = const.tile([P, n], _BF16)             # bias on all partitions
            scale_half = const.tile([P, 1], _F32)
            factor = const.tile([P, 1], _F32)

            # GEMM-side SBUF pools allocated BEFORE phase A so they don't
            # reuse phase A's space (space reuse adds a false dependency on
            # the whole phase-A pool release, serializing wq prefetch and the
            # first matmuls behind the last quantize).
            kxn_pool = ctx.enter_context(
                tc.tile_pool(name="kxn_pool", bufs=k_tiles + 5)
            )
            out_pool = ctx.enter_context(tc.tile_pool(name="out_pool", bufs=3))

            # ---------------- Phase A: load, bf16-cast, absmax ----------------
            with tc.tile_pool(name="phA", bufs=2) as pA:
                xb = pA.tile([P, ks_tot, m_c], _BF16, bufs=1)
                pmax_all = pA.tile([P, k_tiles], _F32, bufs=1)
                for kt in range(k_tiles):
                    xs = pA.tile([P, K_SUB, m_c], _F32, tag="xstage")
                    nc.sync.dma_start(xs[:], xT3[:, ts(kt, K_SUB), :])
                    nc.scalar.activation(
                        xb[:, ts(kt, K_SUB), :], xs[:],
                        mybir.ActivationFunctionType.Copy,
                    )
                    nc.vector.reduce_max(
                        pmax_all[:, ts(kt, 1)],
                        xb[:, ts(kt, K_SUB), :],
                        axis=mybir.AxisListType.XY,
                        apply_absolute_value=True,
                    )
                pmax = pA.tile([P, 1], _F32, bufs=1)
                nc.vector.reduce_max(
                    pmax[:], pmax_all[:], axis=mybir.AxisListType.X
                )

                # cross-partition max via DRAM bounce
                pmax_dram = dram.tile([P], _F32)
                nc.sync.dma_start(pmax_dram[:], pmax[:, 0])
                pmax_row = pA.tile([1, P], _F32, bufs=1)
                nc.sync.dma_start(pmax_row[:], pmax_dram[None, :])
                amax_l = pA.tile([1, 1], _F32, bufs=1)
                nc.vector.reduce_max(
                    amax_l[:], pmax_row[:], axis=mybir.AxisListType.X
                )

                # cross-core AllReduce(max)
                cc_in = dram.tile([1], _F32)
                cc_out = dram.tile([1], _F32, addr_space="Shared")
                nc.sync.dma_start(cc_in[:], amax_l[0, :])
                nc.gpsimd.collective_compute(
                    "AllReduce",
                    mybir.AluOpType.max,
                    ins=[cc_in[:]],
                    outs=[cc_out[:]],
                    replica_groups=[list(range(N_CORES))],
                )
                amax = pA.tile([P, 1], _F32, bufs=1)
                nc.sync.dma_start(amax[:], cc_out[None, :].to_broadcast((P, 1)))

                # scalar chain (replicated across partitions)
                rcp = pA.tile([P, 1], _F32, bufs=1)
                nc.vector.reciprocal(rcp[:], amax[:])
                scale_t = pA.tile([P, 1], _F32, bufs=1)
                nc.vector.tensor_scalar_mul(scale_t[:], rcp[:], F8_MAX)
                nc.vector.tensor_scalar_mul(scale_half[:], scale_t[:], 0.5)
                inv_s = pA.tile([P, 1], _F32, bufs=1)
                nc.vector.reciprocal(inv_s[:], scale_t[:])
                inv_b = pA.tile([P, 1], _BF16, bufs=1)
                nc.vector.tensor_copy(inv_b[:], inv_s[:])
                ws = pA.tile([P, 1], _F32, bufs=1)
                nc.sync.dma_start(ws[:], wscale_d[None, :].to_broadcast((P, 1)))
                ws_b = pA.tile([P, 1], _BF16, bufs=1)
                nc.vector.tensor_copy(ws_b[:], ws[:])
                prod = pA.tile([P, 1], _F32, bufs=1)
                nc.vector.tensor_tensor(
                    prod[:], inv_b[:], ws_b[:], mybir.AluOpType.mult
                )
                nc.vector.tensor_scalar_mul(factor[:], prod[:], 4.0)

                # quantize: qx = fp8(xb * scale/2); both engines compute
                # x*scale in fp32 internally and RNE-cast to fp8 on write.
                for kt in range(k_tiles):
                    if kt % 2 == 0:
                        nc.scalar.activation(
                            qx_blocks[kt][:],
                            xb[:, ts(kt, K_SUB), :],
                            mybir.ActivationFunctionType.Copy,
                            scale=scale_half[:],
                        )
                    else:
                        nc.vector.tensor_tensor(
                            qx_blocks[kt][:],
                            xb[:, ts(kt, K_SUB), :],
                            scale_half[:, None].to_broadcast((P, K_SUB, m_c)),
                            mybir.AluOpType.mult,
                        )

                # bias broadcast: emitted late so x loads win the DMA queues
                nc.sync.dma_start(
                    bias_bc[:], bias_d[None, :].to_broadcast((P, n))
                )

            # ---------------- Phase B: fp8 DoubleRow GEMM ----------------
            kxm_shape = ShapeInfo(pdims=((P, ks_tot),), fdims=(m_c,))

            def kxm_producer(nc_, md):
                assert md.k_batch_idx == 0 and md.m_batch_idx == 0
                assert md.k_subtiles == K_SUB
                return qx_blocks[md.k_tile_idx][
                    :, :, ds(md.m_tile_idx * md.m_tile, md.m_tile)
                ]

            kxn_producer, kxn_shape = dma_from_dram_kxn(kxn_pool, wq[:])

            def mxn_producer(nc_, md):
                prod_tile = out_pool.tile(
                    [min(P, md.m_tile), md.m_subtiles, md.n_tile],
                    _BF16,
                    name="mxn_out",
                    tag="mxn_out",
                )
                return prod_tile

            def reducer(nc_, psum, sbuf, md):
                start = md.n_tile_idx * md.n_tile + md.n_subtile_idx * md.n_subtile
                sz = md.n_slice_size
                nc_.scalar.activation(
                    sbuf[:, :, :sz],
                    psum[:, :sz],
                    mybir.ActivationFunctionType.Copy,
                    scale=factor[: psum.shape[0]],
                )
                nc_.vector.tensor_tensor(
                    sbuf[:, :, :sz],
                    sbuf[:, :, :sz],
                    bias_bc[: psum.shape[0], ds(start, sz)],
                    mybir.AluOpType.add,
                )

            composable_matmul_tile_kernel(
                tc=tc,
                kxm_shape=kxm_shape,
                kxn_shape=kxn_shape,
                output_type=_BF16,
                kxm_producer=kxm_producer,
                kxn_producer=kxn_producer,
                mxn_consumer=dma_to_dram_mxn(out_d[:]),
                mxn_subtile_reducer=reducer,
                mxn_subtile_producer=mxn_producer,
                MATMUL_FREE_DIM=512,
                MAX_TILE_SIZE=512,
                MAX_K_TILE_SIZE=K_TILE,
                cache_tiles=True,
                temps_n_bufs=3,
                psum_n_bufs=2,
            )

    _legalize_waits(nc)
    return nc


# ----------------------------------------------------------------------------
_NC_CACHE = {}


def _get_nc(m_c=M_C, k=K, n=N):
    key = (m_c, k, n)
    if key not in _NC_CACHE:
        _NC_CACHE[key] = build_nc(m_c, k, n)
    return _NC_CACHE[key]


def prepare_in_maps(x, weight, weight_scale, bias, m_c=M_C, n_cores=N_CORES):
    m = x.shape[0] * x.shape[1] if x.ndim == 3 else x.shape[0]
    k = x.shape[-1]
    x2 = np.ascontiguousarray(np.asarray(x, dtype=np.float32).reshape(m, k))
    xT = x2.T  # [k, m] view
    wq = np.ascontiguousarray(
        (np.asarray(weight, dtype=np.float32).T * np.float32(0.5))
    ).astype(ml_dtypes.float8_e4m3)
    bias_np = np.asarray(bias).astype(ml_dtypes.bfloat16)
    ws = np.asarray(weight_scale, dtype=np.float32).reshape(1)
    in_maps = []
    for c in range(n_cores):
        in_maps.append(
            {
                "xT": np.ascontiguousarray(xT[:, c * m_c : (c + 1) * m_c]),
                "wq": wq,
                "bias": bias_np,
                "wscale": ws,
            }
        )
    return in_maps


def kernel(x, weight, weight_scale, bias):
    nc = _get_nc()
    in_maps = prepare_in_maps(x, weight, weight_scale, bias)
    res = run_bass_kernel_spmd(nc, in_maps, list(range(N_CORES)))
    out = np.concatenate([res.results[c]["out"] for c in range(N_CORES)], axis=0)
    return out.reshape(B, S, N)


# revision 11
# speedup vs baseline: 52.6047x; 52.6047x over previous
"""FP8DynamicLinear Trainium2 kernel (8-core SPMD).

Reference semantics (nn_FP8DynamicLinear, native_fp8_support=False branch):
    xb      = bf16(x)
    amax    = max(|min(xb)|, |max(xb)|)            # per-tensor, fp32
    scale   = 448 / max(amax, 1e-12)
    qx      = e4m3fn(clip(xb * scale, +-448))
    a       = bf16(qx) * bf16(1/scale)             # bf16 [M, K]
    b       = bf16(weight) * bf16(weight_scale)    # bf16 [N, K]
    out     = a @ b.T + bias                       # bf16 [B, S, N]

Strategy (M-sharded: 1024 tokens per core, weight replicated):
  * Host: transpose x -> xT [K, M] fp32, slice M across 8 cores; transpose
    weight -> wT [K, N].
  * TRN fp8e4 tops out at +-240 (vs e4m3fn's 448) but the grids agree up to
    240, so quantizing at scale/2 (values <= 224) reproduces e4m3fn rounding
    exactly for all |v| >= 2^-6; the 2x is folded into the dequant factor.
  * fp8 fast path (weight values all on the e4m3fn grid, as produced by the
    reference's per-tensor weight quantization): host pre-casts wT*0.5 to TRN
    fp8; device quantizes x to fp8 and runs a DoubleRow fp8 matmul (2 k-tiles
    per instruction), then psum * (4 * bf16(1/scale) * bf16(w_scale)) + bias.
  * bf16 fallback (arbitrary weight): host computes b.T = bf16(wT *
    bf16(w_scale)); device builds a = bf16(qx * bf16(1/scale)) and runs a
    bf16 matmul; psum + bias.
  * amax is reduced on-device per core and combined with an AllReduce(max)
    collective across the 8 cores, matching the reference bit-for-bit.
"""

from contextlib import ExitStack

import numpy as np
import ml_dtypes

import concourse.bass as bass
import concourse.mybir as mybir
import concourse.tile as tile
from concourse.bass import ts, ds
from concourse.bass_utils import run_bass_kernel_spmd
from concourse.kernels.tile_matmul import (
    ShapeInfo,
    composable_matmul_tile_kernel,
    dma_from_dram_kxn,
    dma_to_dram_mxn,
)

P = 128
F8_MAX = 448.0
N_CORES = 8

# Problem shapes (hardcoded per spec)
B, S, K, N = 2, 4096, 4096, 16384
M = B * S              # 8192 tokens
M_C = M // N_CORES     # 1024 tokens per core
K_TILE = 512
K_SUB = K_TILE // P    # 4
K_TILES = K // K_TILE  # 8

_F32 = mybir.dt.float32
_BF16 = mybir.dt.bfloat16
_F8 = mybir.dt.float8e4

# ----------------------------------------------------------------------------
# Wait legalizer: this container's walrus rejects engine instructions with
# more than 1 inline sync-wait (and EventSemaphore with more than 2), but
# bass_rust nop-fusion fuses 2 waits + 1 update into one instruction.  Split
# the excess onto preceding InstEventSemaphore carriers on the same engine.
_EXEMPT = (
    "InstEventSemaphore",
    "InstUnconditionalBranch",
    "InstConditionalBranch",
    "InstCall",
    "InstRegisterMove",
)
_ES_CAP = 2


def _legalize_waits(nc, max_inline=1):
    n_split = 0
    for fn in nc.m.functions:
        for blk in fn.blocks:
            insts = blk.instructions
            i = 0
            while i < len(insts):
                inst = insts[i]
                si = inst.sync_info
                if (
                    si is not None
                    and len(si.on_wait) > max_inline
                    and inst.__class__.__name__ not in _EXEMPT
                ):
                    extra = list(si.on_wait[:-max_inline])
                    keep = list(si.on_wait[-max_inline:])
                    inst.sync_info = mybir.SyncInfo(
                        on_wait=keep, on_update=list(si.on_update)
                    )
                    carriers = [
                        mybir.InstEventSemaphore(
                            name=f"{inst.name}-wsplit{j}",
                            engine=inst.engine,
                            bass_nofuse=True,
                            sync_info=mybir.SyncInfo(
                                on_wait=extra[j : j + _ES_CAP], on_update=[]
                            ),
                        )
                        for j in range(0, len(extra), _ES_CAP)
                    ]
                    for kk, es in enumerate(carriers):
                        insts.insert(i + kk, es)
                    i += len(carriers)
                    n_split += 1
                i += 1
    return n_split


# ----------------------------------------------------------------------------
def build_nc(m_c=M_C, k=K, n=N, mode="fp8"):
    assert mode in ("fp8", "bf16")
    ks_tot = k // P
    k_tiles = k // K_TILE

    nc = bass.Bass()
    xT = nc.dram_tensor("xT", [k, m_c], _F32, kind="ExternalInput")
    wq = nc.dram_tensor("wq", [k, n], _F8 if mode == "fp8" else _BF16,
                        kind="ExternalInput")
    bias_d = nc.dram_tensor("bias", [n], _BF16, kind="ExternalInput")
    wscale_d = nc.dram_tensor("wscale", [1], _F32, kind="ExternalInput")
    out_d = nc.dram_tensor("out", [m_c, n], _BF16, kind="ExternalOutput")

    xT3 = xT.rearrange("(o p) m -> p o m", p=P)  # k = o*128 + p

    with tile.TileContext(nc) as tc:
        with ExitStack() as ctx:
            const = ctx.enter_context(tc.tile_pool(name="const", bufs=1))
            dram = ctx.enter_context(tc.tile_pool(name="dram", bufs=1, space="DRAM"))

            # lhsT blocks: fp8 qx on the fast path, bf16 `a` on the fallback
            lhs_dt = _F8 if mode == "fp8" else _BF16
            lhs_blocks = [
                const.tile([P, K_SUB, m_c], lhs_dt, name=f"lhs{kt}", tag=f"lhs{kt}")
                for kt in range(k_tiles)
            ]
            bias_bc = const.tile([P, n], _BF16)
            scale_half = const.tile([P, 1], _F32)
            factor = const.tile([P, 1], _F32)  # fp8: 4*bf16(1/s)*bf16(ws); bf16: 2*bf16(1/s)

            # GEMM-side SBUF pools allocated BEFORE phase A so they don't
            # reuse phase A's space (space reuse adds a false dependency on
            # the whole phase-A pool release, serializing wq prefetch and
            # the first matmuls behind the last quantize).
            kxn_bufs = (k_tiles + 5) if mode == "fp8" else (k_tiles + 1)
            kxn_pool = ctx.enter_context(
                tc.tile_pool(name="kxn_pool", bufs=kxn_bufs)
            )
            out_pool = ctx.enter_context(tc.tile_pool(name="out_pool", bufs=3))
            if mode == "bf16":
                qtmp_pool = ctx.enter_context(tc.tile_pool(name="qtmp", bufs=2))

            # ---------------- Phase A: load, bf16-cast, absmax ----------------
            with tc.tile_pool(name="phA", bufs=2) as pA:
                # fp8 mode keeps the bf16 cast resident for the quantize
                # pass; bf16 mode re-streams x instead (SBUF is tight there).
                if mode == "fp8":
                    xb = pA.tile([P, ks_tot, m_c], _BF16, bufs=1)
                pmax_all = pA.tile([P, k_tiles], _F32, bufs=1)
                for kt in range(k_tiles):
                    xs = pA.tile([P, K_SUB, m_c], _F32, tag="xstage")
                    nc.sync.dma_start(xs[:], xT3[:, ts(kt, K_SUB), :])
                    if mode == "fp8":
                        xbt = xb[:, ts(kt, K_SUB), :]
                    else:
                        xbt = pA.tile([P, K_SUB, m_c], _BF16, tag="xbt")
                    nc.scalar.activation(
                        xbt, xs[:],
                        mybir.ActivationFunctionType.Copy,
                    )
                    nc.vector.reduce_max(
                        pmax_all[:, ts(kt, 1)],
                        xbt,
                        axis=mybir.AxisListType.XY,
                        apply_absolute_value=True,
                    )
                pmax = pA.tile([P, 1], _F32, bufs=1)
                nc.vector.reduce_max(
                    pmax[:], pmax_all[:], axis=mybir.AxisListType.X
                )

                # cross-partition max via DRAM bounce
                pmax_dram = dram.tile([P], _F32)
                nc.sync.dma_start(pmax_dram[:], pmax[:, 0])
                pmax_row = pA.tile([1, P], _F32, bufs=1)
                nc.sync.dma_start(pmax_row[:], pmax_dram[None, :])
                amax_l = pA.tile([1, 1], _F32, bufs=1)
                nc.vector.reduce_max(
                    amax_l[:], pmax_row[:], axis=mybir.AxisListType.X
                )

                # cross-core AllReduce(max)
                cc_in = dram.tile([1], _F32)
                cc_out = dram.tile([1], _F32, addr_space="Shared")
                nc.sync.dma_start(cc_in[:], amax_l[0, :])
                nc.gpsimd.collective_compute(
                    "AllReduce",
                    mybir.AluOpType.max,
                    ins=[cc_in[:]],
                    outs=[cc_out[:]],
                    replica_groups=[list(range(N_CORES))],
                )
                amax = pA.tile([P, 1], _F32, bufs=1)
                nc.sync.dma_start(amax[:], cc_out[None, :].to_broadcast((P, 1)))

                # scalar chain (replicated on all partitions):
                # scale = 448/max(amax, 1e-12); scale_half = scale/2
                nc.vector.tensor_scalar_max(amax[:], amax[:], 1e-12)
                rcp = pA.tile([P, 1], _F32, bufs=1)
                nc.vector.reciprocal(rcp[:], amax[:])
                scale_t = pA.tile([P, 1], _F32, bufs=1)
                nc.vector.tensor_scalar_mul(scale_t[:], rcp[:], F8_MAX)
                nc.vector.tensor_scalar_mul(scale_half[:], scale_t[:], 0.5)
                inv_s = pA.tile([P, 1], _F32, bufs=1)
                nc.vector.reciprocal(inv_s[:], scale_t[:])
                inv_b = pA.tile([P, 1], _BF16, bufs=1)
                nc.vector.tensor_copy(inv_b[:], inv_s[:])
                if mode == "fp8":
                    ws = pA.tile([P, 1], _F32, bufs=1)
                    nc.sync.dma_start(
                        ws[:], wscale_d[None, :].to_broadcast((P, 1))
                    )
                    ws_b = pA.tile([P, 1], _BF16, bufs=1)
                    nc.vector.tensor_copy(ws_b[:], ws[:])
                    prod = pA.tile([P, 1], _F32, bufs=1)
                    nc.vector.tensor_tensor(
                        prod[:], inv_b[:], ws_b[:], mybir.AluOpType.mult
                    )
                    nc.vector.tensor_scalar_mul(factor[:], prod[:], 4.0)
                else:
                    # a = bf16(qx_half * factor), factor = 2*bf16(1/s)
                    nc.vector.tensor_scalar_mul(factor[:], inv_b[:], 2.0)

                # quantize: fp8(xb * scale/2); both engines compute x*scale
                # in fp32 internally and RNE-cast to fp8 on the write.
                if mode == "fp8":
                    for kt in range(k_tiles):
                        if kt % 2 == 0:
                            nc.scalar.activation(
                                lhs_blocks[kt][:],
                                xb[:, ts(kt, K_SUB), :],
                                mybir.ActivationFunctionType.Copy,
                                scale=scale_half[:],
                            )
                        else:
                            nc.vector.tensor_tensor(
                                lhs_blocks[kt][:],
                                xb[:, ts(kt, K_SUB), :],
                                scale_half[:, None].to_broadcast((P, K_SUB, m_c)),
                                mybir.AluOpType.mult,
                            )
                else:
                    for kt in range(k_tiles):
                        xs2 = pA.tile([P, K_SUB, m_c], _F32, tag="xstage")
                        nc.sync.dma_start(xs2[:], xT3[:, ts(kt, K_SUB), :])
                        xbt2 = pA.tile([P, K_SUB, m_c], _BF16, tag="xbt")
                        nc.scalar.activation(
                            xbt2[:], xs2[:],
                            mybir.ActivationFunctionType.Copy,
                        )
                        qt = qtmp_pool.tile([P, K_SUB, m_c], _F8, tag="qtmp")
                        nc.vector.tensor_tensor(
                            qt[:],
                            xbt2[:],
                            scale_half[:, None].to_broadcast((P, K_SUB, m_c)),
                            mybir.AluOpType.mult,
                        )
                        nc.scalar.activation(
                            lhs_blocks[kt][:],
                            qt[:],
                            mybir.ActivationFunctionType.Copy,
                            scale=factor[:],
                        )

                # bias broadcast: emitted late so x loads win the DMA queues
                nc.sync.dma_start(
                    bias_bc[:], bias_d[None, :].to_broadcast((P, n))
                )

            # ---------------- Phase B: matmul ----------------
            kxm_shape = ShapeInfo(pdims=((P, ks_tot),), fdims=(m_c,))

            def kxm_producer(nc_, md):
                assert md.k_batch_idx == 0 and md.m_batch_idx == 0
                assert md.k_subtiles == K_SUB
                return lhs_blocks[md.k_tile_idx][
                    :, :, ds(md.m_tile_idx * md.m_tile, md.m_tile)
                ]

            kxn_producer, kxn_shape = dma_from_dram_kxn(kxn_pool, wq[:])

            def mxn_producer(nc_, md):
                prod_tile = out_pool.tile(
                    [min(P, md.m_tile), md.m_subtiles, md.n_tile],
                    _BF16,
                    name="mxn_out",
                    tag="mxn_out",
                )
                return prod_tile

            if mode == "fp8":
                def reducer(nc_, psum, sbuf, md):
                    start = (md.n_tile_idx * md.n_tile
                             + md.n_subtile_idx * md.n_subtile)
                    sz = md.n_slice_size
                    nc_.scalar.activation(
                        sbuf[:, :, :sz],
                        psum[:, :sz],
                        mybir.ActivationFunctionType.Copy,
                        scale=factor[: psum.shape[0]],
                    )
                    nc_.vector.tensor_tensor(
                        sbuf[:, :, :sz],
                        sbuf[:, :, :sz],
                        bias_bc[: psum.shape[0], ds(start, sz)],
                        mybir.AluOpType.add,
                    )
            else:
                def reducer(nc_, psum, sbuf, md):
                    start = (md.n_tile_idx * md.n_tile
                             + md.n_subtile_idx * md.n_subtile)
                    sz = md.n_slice_size
                    nc_.vector.tensor_tensor(
                        sbuf[:, :, :sz],
                        psum[:, :sz],
                        bias_bc[: psum.shape[0], ds(start, sz)],
                        mybir.AluOpType.add,
                    )

            composable_matmul_tile_kernel(
                tc=tc,
                kxm_shape=kxm_shape,
                kxn_shape=kxn_shape,
                output_type=_BF16,
                kxm_producer=kxm_producer,
                kxn_producer=kxn_producer,
                mxn_consumer=dma_to_dram_mxn(out_d[:]),
                mxn_subtile_reducer=reducer,
                mxn_subtile_producer=mxn_producer,
                MATMUL_FREE_DIM=512,
                MAX_TILE_SIZE=512,
                MAX_K_TILE_SIZE=K_TILE,
                cache_tiles=True,
                temps_n_bufs=3,
                psum_n_bufs=2,
            )

    _legalize_waits(nc)
    return nc


# ----------------------------------------------------------------------------
_NC_CACHE = {}


def _get_nc(m_c=M_C, k=K, n=N, mode="fp8"):
    key = (m_c, k, n, mode)
    if key not in _NC_CACHE:
        _NC_CACHE[key] = build_nc(m_c, k, n, mode)
    return _NC_CACHE[key]


def _weight_is_fp8_grid(w):
    """True iff every weight value survives the *0.5 -> TRN fp8e4 round-trip
    exactly (i.e. the tensor is on the e4m3fn grid, like the reference's
    pre-quantized weight)."""
    flat = w.reshape(-1)
    idx = np.linspace(0, flat.size - 1, 4096, dtype=np.int64)
    s = flat[idx] * np.float32(0.5)
    rt = s.astype(ml_dtypes.float8_e4m3).astype(np.float32)
    if not np.array_equal(rt, s):
        return False
    half = w * np.float32(0.5)
    rt = half.astype(ml_dtypes.float8_e4m3).astype(np.float32)
    return np.array_equal(rt, half)


def prepare_in_maps(x, weight, weight_scale, bias, m_c=M_C, n_cores=N_CORES,
                    mode="fp8"):
    m = x.shape[0] * x.shape[1] if x.ndim == 3 else x.shape[0]
    k = x.shape[-1]
    x2 = np.ascontiguousarray(np.asarray(x, dtype=np.float32).reshape(m, k))
    xT = x2.T  # [k, m] view
    wT = np.ascontiguousarray(np.asarray(weight, dtype=np.float32).T)
    ws = np.asarray(weight_scale, dtype=np.float32).reshape(1)
    if mode == "fp8":
        wq = (wT * np.float32(0.5)).astype(ml_dtypes.float8_e4m3)
    else:
        # b.T = bf16(wT * bf16(w_scale)) -- exactly the reference's b
        ws_b = np.float32(ws[0].astype(ml_dtypes.bfloat16))
        wq = (wT * ws_b).astype(ml_dtypes.bfloat16)
    bias_np = np.asarray(bias).astype(ml_dtypes.bfloat16)
    in_maps = []
    for c in range(n_cores):
        in_maps.append(
            {
                "xT": np.ascontiguousarray(xT[:, c * m_c : (c + 1) * m_c]),
                "wq": wq,
                "bias": bias_np,
                "wscale": ws,
            }
        )
    return in_maps


def kernel(x, weight, weight_scale, bias):
    w = np.asarray(weight, dtype=np.float32)
    mode = "fp8" if _weight_is_fp8_grid(w) else "bf16"
    nc = _get_nc(mode=mode)
    in_maps = prepare_in_maps(x, w, weight_scale, bias, mode=mode)
    res = run_bass_kernel_spmd(nc, in_maps, list(range(N_CORES)))
    out = np.concatenate([res.results[c]["out"] for c in range(N_CORES)], axis=0)
    return out.reshape(B, S, N)
